# revision 2
# baseline (speedup 1.0000x reference)
"""Trainium2 Bass kernel for the CharRNN (2-layer GRU + adaptive softmax) loss.

v2 over the baseline:
  - Software-pipelined GRU emission: PE order per iteration is
    pg1(t), pg2(t-1), pc1(t), pc2(t-1), proj(t-2) so the vector/scalar
    latency of each gate evacuation hides under the other layer's large
    gate matmul block (the PE is instruction-dispatch bound at ~34ns/matmul).
  - Embedding prologue interleaved with the GRU (2-tile lookahead) and the
    transposed embeddings kept SBUF-resident ([128, 2, NT] bf16).
  - Adaptive softmax: 3 slots per core covering tiles 0..20, plus tiles
    21..24 handled as a shared epilogue whose tail vocab is split across
    all 8 cores via per-core wtail_ep slices; the host sums the partial
    exp-sums and finishes those 512 tokens' losses.
  - Tail-chunk work is paced (budget per GRU iteration) so the scalar
    engine's exp never backs up the PE through PSUM pressure.
"""

import sys
import types

sys.path.insert(0, "/opt/trn_rl_repo")

import numpy as np
import ml_dtypes


def _install_ntff_hook():
    if "antenv.axon_hooks" in sys.modules:
        return
    try:
        from trn_agent_boot.trn_boot import _ntff_profile_via_ctypes
        hook = _ntff_profile_via_ctypes("/opt/axon/libaxon_pjrt.so")
    except Exception:
        hook = None
    mod = types.ModuleType("antenv.axon_hooks")
    mod.get_axon_ntff_profile_hook = lambda: hook
    mod.set_axon_ntff_profile_hook = lambda h: None
    sys.modules["antenv.axon_hooks"] = mod


_install_ntff_hook()

import concourse.bass as bass
import concourse.bacc as bacc_mod
import concourse.mybir as mybir
import concourse.tile as tile
from concourse.bass import ts
from concourse.bass_utils import run_bass_kernel_spmd
from concourse.masks import make_identity

F32 = mybir.dt.float32
BF16 = mybir.dt.bfloat16
FP8 = mybir.dt.float8e4
I32 = mybir.dt.int32
AL = mybir.AluOpType
AF = mybir.ActivationFunctionType

V, B, T, R, U = 32000, 64, 50, 1024, 256
CUT, TAILP = 2000, 64
NT = B * T
NCORES = 8
NSLOT = 3             # in-loop softmax slots per core (tiles 0..20)
NEP = 4               # shared epilogue tiles 21..24
EP_T0 = 21
EPW = 4096            # per-core epilogue tail vocab slice (32768 / 8)
TPAD_EP = 32768
NTILE = NT // 128     # 25
HPAD = 2048
TPAD = 30720
KG1 = (U + R) // 128  # 10
KG2 = (2 * R) // 128  # 16
WSCALE = 16.0
CHUNK_BUDGET = 12     # softmax work units per GRU iteration


def _bank_start(m, k):
    return k == 0 and (m % 8) == 0


def _bank_stop(m, k, n_m, n_k):
    return (m % 8 == 7 or m == n_m - 1) and k == n_k - 1


def build_program():
    nc = bacc_mod.Bacc()
    dp = nc.declare_dram_parameter

    ids_e = dp("ids_sb", [128, NTILE], I32, isOutput=False)
    emb_e = dp("emb", [V, U], BF16, isOutput=False)
    wg1_e = dp("wg1", [128, KG1, 2 * R], FP8, isOutput=False)
    wc1_e = dp("wc1", [128, KG1, R], FP8, isOutput=False)
    wg2_e = dp("wg2", [128, KG2, 2 * R], FP8, isOutput=False)
    wc2_e = dp("wc2", [128, KG2, R], FP8, isOutput=False)
    wp_e = dp("wp", [128, R // 128, U], BF16, isOutput=False)
    bg1_e = dp("bg1", [128, 16], F32, isOutput=False)
    bc1_e = dp("bc1", [128, 8], F32, isOutput=False)
    bg2_e = dp("bg2", [128, 16], F32, isOutput=False)
    bc2_e = dp("bc2", [128, 8], F32, isOutput=False)
    bp_e = dp("bp", [128, 2], F32, isOutput=False)
    whead_e = dp("whead", [128, 2, HPAD], BF16, isOutput=False)
    wtail_e = dp("wtail", [64, TPAD], FP8, isOutput=False)
    wtailep_e = dp("wtail_ep", [64, EPW], FP8, isOutput=False)
    wtp_e = dp("wtp", [128, 2, TAILP], BF16, isOutput=False)
    wheadT_e = dp("wheadT", [CUT + 1, U], F32, isOutput=False)
    wtailT_e = dp("wtailT", [V - CUT, TAILP], F32, isOutput=False)
    tok_e = dp("tok_idx", [128, NSLOT], I32, isOutput=False)
    hd_e = dp("hd_idx", [128, NSLOT], I32, isOutput=False)
    tl_e = dp("tl_idx", [128, NSLOT], I32, isOutput=False)
    mt_e = dp("mtail", [128, NSLOT], F32, isOutput=False)
    vl_e = dp("valid", [128, NSLOT], F32, isOutput=False)
    eptok_e = dp("ep_tok", [128, NEP], I32, isOutput=False)
    ephd_e = dp("ep_hd", [128, NEP], I32, isOutput=False)
    eptl_e = dp("ep_tl", [128, NEP], I32, isOutput=False)
    loss_e = dp("loss_sum", [1, 1], F32, isOutput=True)
    eptsum_e = dp("ep_tsum", [128, NEP], F32, isOutput=True)
    ephx_e = dp("ep_hx", [128, 3 * NEP], F32, isOutput=True)

    orow_d = nc.dram_tensor("orow_d", [NT, U], BF16)

    with tile.TileContext(nc) as tc:
        with tc.tile_pool(name="persist", bufs=1) as P:
            ids_sb = P.tile([128, NTILE], I32)
            nc.sync.dma_start(out=ids_sb[:], in_=ids_e[:])
            idf = P.tile([128, 128], F32)
            make_identity(nc, idf[:])
            idb = P.tile([128, 128], BF16)
            nc.vector.tensor_copy(out=idb[:], in_=idf[:])

            embT = P.tile([128, 2, NT], BF16)

            bg1 = P.tile([128, 16], F32)
            bc1 = P.tile([128, 8], F32)
            bg2 = P.tile([128, 16], F32)
            bc2 = P.tile([128, 8], F32)
            bp = P.tile([128, 2], F32)
            tok_i = P.tile([128, NSLOT], I32)
            hd_i = P.tile([128, NSLOT], I32)
            tl_i = P.tile([128, NSLOT], I32)
            mt_m = P.tile([128, NSLOT], F32)
            vl_m = P.tile([128, NSLOT], F32)
            ep_tok = P.tile([128, NEP], I32)
            ep_hd = P.tile([128, NEP], I32)
            ep_tl = P.tile([128, NEP], I32)
            for dst, src in ((bg1, bg1_e), (bc1, bc1_e), (bg2, bg2_e),
                             (bc2, bc2_e), (bp, bp_e), (tok_i, tok_e),
                             (hd_i, hd_e), (tl_i, tl_e), (mt_m, mt_e),
                             (vl_m, vl_e), (ep_tok, eptok_e),
                             (ep_hd, ephd_e), (ep_tl, eptl_e)):
                nc.sync.dma_start(out=dst[:], in_=src[:])

            hsums = P.tile([128, NSLOT, HPAD // 512], F32)
            tsums = P.tile([128, NSLOT, TPAD // 512], F32)
            ep_hsums = P.tile([128, NEP, HPAD // 512], F32)
            ep_tsums = P.tile([128, NEP, EPW // 512], F32)
            hsr = P.tile([128, NSLOT], F32)
            tsr = P.tile([128, NSLOT], F32)
            ep_hsr = P.tile([128, NEP], F32)
            lzh = P.tile([128, NSLOT], F32)
            lzt = P.tile([128, NSLOT], F32)
            xhd = P.tile([128, NSLOT], F32)
            xtl = P.tile([128, NSLOT], F32)
            ep_hx = P.tile([128, 3 * NEP], F32)
            ep_ts = P.tile([128, NEP], F32)
            loss_t = P.tile([128, NSLOT], F32)
            ones = P.tile([128, 1], F32)
            nc.gpsimd.memset(ones[:], 1.0)
            hpadc = P.tile([128, 1], F32)
            nc.gpsimd.memset(hpadc[:], -float(HPAD - (CUT + 1)))
            tpadc = P.tile([128, 1], F32)
            nc.gpsimd.memset(tpadc[:], -float(TPAD - (V - CUT)))

            with tc.tile_pool(name="wpool", bufs=1) as W, \
                 tc.tile_pool(name="gru", bufs=2) as GR, \
                 tc.tile_pool(name="smw", bufs=2) as SW, \
                 tc.tile_pool(name="gps", bufs=2, space="PSUM") as PP:
                wg1 = W.tile([128, KG1, 2 * R], FP8)
                wc1 = W.tile([128, KG1, R], FP8)
                wg2 = W.tile([128, KG2, 2 * R], FP8)
                wc2 = W.tile([128, KG2, R], FP8)
                wp = W.tile([128, R // 128, U], BF16)
                nc.sync.dma_start(out=wg1[:, :, 0:R], in_=wg1_e[:, :, 0:R])
                nc.sync.dma_start(out=wg1[:, :, R:2 * R],
                                  in_=wg1_e[:, :, R:2 * R])
                nc.sync.dma_start(out=wc1[:], in_=wc1_e[:])
                nc.sync.dma_start(out=wg2[:, :, 0:R], in_=wg2_e[:, :, 0:R])
                nc.sync.dma_start(out=wg2[:, :, R:2 * R],
                                  in_=wg2_e[:, :, R:2 * R])
                nc.sync.dma_start(out=wc2[:], in_=wc2_e[:])
                nc.sync.dma_start(out=wp[:], in_=wp_e[:])
                whead = W.tile([128, 2, HPAD], BF16)
                wtp = W.tile([128, 2, TAILP], BF16)
                wtail = W.tile([64, TPAD], FP8)
                wtail_ep = W.tile([64, EPW], FP8)
                smw_loaded = [False]

                def load_sm_weights():
                    if smw_loaded[0]:
                        return
                    smw_loaded[0] = True
                    nc.sync.dma_start(out=whead[:], in_=whead_e[:])
                    nc.sync.dma_start(out=wtp[:], in_=wtp_e[:])
                    nc.sync.dma_start(out=wtail[:], in_=wtail_e[:])
                    nc.sync.dma_start(out=wtail_ep[:], in_=wtailep_e[:])

                # ---------------- embedding prologue tile (interleaved) ----
                def emit_emb_tile(i):
                    et = GR.tile([128, U], BF16, tag="et", bufs=3)
                    nc.gpsimd.indirect_dma_start(
                        out=et[:], out_offset=None, in_=emb_e[:],
                        in_offset=bass.IndirectOffsetOnAxis(
                            ap=ids_sb[:, i:i + 1], axis=0))
                    for k in range(2):
                        pt = PP.tile([128, 128], BF16, tag="smb", space="PSUM")
                        nc.tensor.transpose(
                            out=pt[:], in_=et[:, k * 128:(k + 1) * 128],
                            identity=idb[:])
                        nc.vector.tensor_copy(
                            out=embT[:, k, i * 128:(i + 1) * 128], in_=pt[:])

                # ---------------- GRU cell pieces ---------------------------
                h1 = GR.tile([128, 8, 64], BF16, tag="h1", bufs=3)
                h2 = GR.tile([128, 8, 64], BF16, tag="h2", bufs=3)
                nc.vector.memset(h1[:], 0.0)
                nc.vector.memset(h2[:], 0.0)

                def mm_block(psum_ap, wt, n_k, n_m, rhs_of_k):
                    for m in range(n_m):
                        for k in range(n_k):
                            nc.tensor.matmul(
                                out=psum_ap[:, m * 64:(m + 1) * 64],
                                lhsT=wt[:, k, m * 128:(m + 1) * 128],
                                rhs=rhs_of_k(k),
                                start=_bank_start(m, k),
                                stop=_bank_stop(m, k, n_m, n_k))

                def gates_mm(wg, n_k, rhs_of_k, tag):
                    pg = PP.tile([128, 1024], F32, tag="pg", space="PSUM",
                                 name=f"pg_{tag}")
                    mm_block(pg, wg, n_k, 16, rhs_of_k)
                    return pg

                def gates_post_r(pg, bgt, hprev, tag):
                    sgr = GR.tile([128, 8, 64], BF16, tag="scr16r")
                    nc.vector.scalar_tensor_tensor(
                        out=sgr[:],
                        in0=pg[:, 0:512].rearrange("p (m b) -> p m b", b=64),
                        scalar=1.0 / WSCALE,
                        in1=bgt[:, 0:8].to_broadcast([128, 8, 64]),
                        op0=AL.mult, op1=AL.add)
                    gr = GR.tile([128, 8, 64], BF16, tag="g16r")
                    nc.scalar.activation(out=gr[:], in_=sgr[:], func=AF.Sigmoid)
                    rh = GR.tile([128, 8, 64], BF16, tag="rh")
                    nc.vector.tensor_mul(out=rh[:], in0=gr[:], in1=hprev[:])
                    return rh

                def gates_post_u(pg, bgt, tag):
                    sgu = GR.tile([128, 8, 64], BF16, tag="scr16u")
                    nc.vector.scalar_tensor_tensor(
                        out=sgu[:],
                        in0=pg[:, 512:1024].rearrange("p (m b) -> p m b", b=64),
                        scalar=1.0 / WSCALE,
                        in1=bgt[:, 8:16].to_broadcast([128, 8, 64]),
                        op0=AL.mult, op1=AL.add)
                    gu = GR.tile([128, 8, 64], BF16, tag="g16u")
                    nc.scalar.activation(out=gu[:], in_=sgu[:], func=AF.Sigmoid)
                    return gu

                def cand_mm(wc, n_k, rhs_of_k, tag):
                    pc = PP.tile([128, 512], F32, tag="pc", space="PSUM",
                                 name=f"pc_{tag}")
                    mm_block(pc, wc, n_k, 8, rhs_of_k)
                    return pc

                def cand_post(pc, bct, g, hprev, htag):
                    sc = GR.tile([128, 8, 64], BF16, tag="scr8")
                    nc.vector.scalar_tensor_tensor(
                        out=sc[:], in0=pc[:].rearrange("p (m b) -> p m b", b=64),
                        scalar=1.0 / WSCALE,
                        in1=bct[:].to_broadcast([128, 8, 64]),
                        op0=AL.mult, op1=AL.add)
                    c = GR.tile([128, 8, 64], BF16, tag="c8")
                    nc.scalar.activation(out=c[:], in_=sc[:], func=AF.Tanh)
                    t1 = GR.tile([128, 8, 64], BF16, tag="tt")
                    nc.vector.tensor_sub(out=t1[:], in0=hprev[:], in1=c[:])
                    t2 = GR.tile([128, 8, 64], BF16, tag="tt2")
                    nc.vector.tensor_mul(out=t2[:], in0=g[:], in1=t1[:])
                    hn = GR.tile([128, 8, 64], BF16, tag=htag, bufs=3)
                    nc.vector.tensor_add(out=hn[:], in0=c[:], in1=t2[:])
                    return hn

                def emit_proj_mm(t, h2t):
                    po = PP.tile([128, 512], F32, tag="pc", space="PSUM",
                                 name=f"po_{t}")
                    for m in range(2):
                        for k in range(8):
                            nc.tensor.matmul(
                                out=po[:, m * 64:(m + 1) * 64],
                                lhsT=wp[:, k, m * 128:(m + 1) * 128],
                                rhs=h2t[:, k, :],
                                start=(m == 0 and k == 0),
                                stop=(m == 1 and k == 7))
                    ot = GR.tile([128, 2, 64], BF16, tag="ot", bufs=3)
                    nc.vector.tensor_tensor(
                        out=ot[:],
                        in0=po[:, 0:128].rearrange("p (m b) -> p m b", b=64),
                        in1=bp[:].to_broadcast([128, 2, 64]), op=AL.add)
                    return ot

                def emit_proj_tr(t, ot):
                    orow = GR.tile([64, U], BF16, tag="orow")
                    for k in range(2):
                        ptr = PP.tile([128, 128], BF16, tag="smb", space="PSUM",
                                      name=f"ptr_{t}_{k}")
                        nc.tensor.transpose(
                            out=ptr[:64, :128], in_=ot[:, k, :], identity=idb[:])
                        nc.vector.tensor_copy(
                            out=orow[:, k * 128:(k + 1) * 128], in_=ptr[:64, :128])
                    nc.sync.dma_start(out=orow_d[ts(t, 64), :], in_=orow[:])

                # ---------------- softmax slot / epilogue pieces ------------
                slot_state = {}

                def sm_prep(s, tok_ap, tag):
                    orows = SW.tile([128, U], BF16, tag="orows", bufs=3,
                                    name=f"orows_{tag}")
                    nc.gpsimd.indirect_dma_start(
                        out=orows[:], out_offset=None, in_=orow_d[:],
                        in_offset=bass.IndirectOffsetOnAxis(
                            ap=tok_ap, axis=0))
                    oT = SW.tile([128, 2, 128], BF16, tag="oT", bufs=3,
                                 name=f"oT_{tag}")
                    for k in range(2):
                        ptr = PP.tile([128, 128], BF16, tag="smb", space="PSUM",
                                      name=f"smtr_{tag}_{k}")
                        nc.tensor.transpose(
                            out=ptr[:], in_=orows[:, k * 128:(k + 1) * 128],
                            identity=idb[:])
                        nc.vector.tensor_copy(out=oT[:, k, :], in_=ptr[:])
                    return orows, oT

                def sm_head(oT, hs_ap, lzh_ap, tag):
                    for g in range(HPAD // 512):
                        ph = PP.tile([128, 512], F32, tag="smb", space="PSUM",
                                     name=f"ph_{tag}_{g}")
                        for k in range(2):
                            nc.tensor.matmul(
                                out=ph[:], lhsT=oT[:, k, :],
                                rhs=whead[:, k, g * 512:(g + 1) * 512],
                                start=(k == 0), stop=(k == 1))
                        esc = SW.tile([128, 512], BF16, tag="esc",
                                      name=f"esc_{tag}_{g}")
                        nc.scalar.activation(
                            out=esc[:], in_=ph[:], func=AF.Exp,
                            accum_out=hs_ap[:, g:g + 1])
                    nc.vector.tensor_reduce(
                        out=lzh_ap, in_=hs_ap[:], op=AL.add,
                        axis=mybir.AxisListType.X)

                def sm_xhd(orows, hd_ap, xhd_ap, tag):
                    whs = SW.tile([128, U], F32, tag="whs", name=f"whs_{tag}")
                    nc.gpsimd.indirect_dma_start(
                        out=whs[:], out_offset=None, in_=wheadT_e[:],
                        in_offset=bass.IndirectOffsetOnAxis(ap=hd_ap, axis=0))
                    orf = SW.tile([128, U], F32, tag="orf", name=f"orf_{tag}")
                    nc.vector.tensor_copy(out=orf[:], in_=orows[:])
                    dsc = SW.tile([128, U], F32, tag="dsc", name=f"dsc_{tag}")
                    nc.vector.tensor_mul(out=dsc[:], in0=orf[:], in1=whs[:])
                    nc.vector.tensor_reduce(
                        out=xhd_ap, in_=dsc[:], op=AL.add,
                        axis=mybir.AxisListType.X)

                def sm_tailprep(oT, tag):
                    ppr = PP.tile([128, 512], F32, tag="smb", space="PSUM",
                                  name=f"ppr_{tag}")
                    for k in range(2):
                        nc.tensor.matmul(
                            out=ppr[:, 0:TAILP], lhsT=oT[:, k, :],
                            rhs=wtp[:, k, :], start=(k == 0), stop=(k == 1))
                    prow = SW.tile([128, TAILP], F32, tag="prow", bufs=3,
                                   name=f"prow_{tag}")
                    nc.vector.tensor_copy(out=prow[:], in_=ppr[:, 0:TAILP])
                    ppt = PP.tile([128, 512], F32, tag="smb", space="PSUM",
                                  name=f"ppt_{tag}")
                    for k in range(2):
                        nc.tensor.matmul(
                            out=ppt[:TAILP, 0:128], lhsT=wtp[:, k, :],
                            rhs=oT[:, k, :], start=(k == 0), stop=(k == 1))
                    pT = SW.tile([64, 128], BF16, tag="pT", bufs=3,
                                 name=f"pT_{tag}")
                    nc.vector.tensor_copy(out=pT[:], in_=ppt[:TAILP, 0:128])
                    return prow, pT

                def sm_chunk(pT, wt_ap, ts_ap, tag):
                    pt_ = PP.tile([128, 512], F32, tag="smb", space="PSUM",
                                  name=f"ptl_{tag}")
                    nc.tensor.matmul(
                        out=pt_[:], lhsT=pT[:], rhs=wt_ap,
                        start=True, stop=True)
                    esc2 = SW.tile([128, 512], BF16, tag="esc",
                                   name=f"esc2_{tag}")
                    nc.scalar.activation(
                        out=esc2[:], in_=pt_[:], func=AF.Exp,
                        accum_out=ts_ap)

                def sm_tailpost(ts_ap, lzt_ap, prow, tl_ap, xtl_ap, tag):
                    nc.vector.tensor_reduce(
                        out=lzt_ap, in_=ts_ap, op=AL.add,
                        axis=mybir.AxisListType.X)
                    wts = SW.tile([128, TAILP], F32, tag="wts",
                                  name=f"wts_{tag}")
                    nc.gpsimd.indirect_dma_start(
                        out=wts[:], out_offset=None, in_=wtailT_e[:],
                        in_offset=bass.IndirectOffsetOnAxis(ap=tl_ap, axis=0))
                    dsc2 = SW.tile([128, TAILP], F32, tag="wts2",
                                   name=f"dsc2_{tag}")
                    nc.vector.tensor_mul(out=dsc2[:], in0=prow[:], in1=wts[:])
                    nc.vector.tensor_reduce(
                        out=xtl_ap, in_=dsc2[:], op=AL.add,
                        axis=mybir.AxisListType.X)

                # ---------------- softmax work scheduler --------------------
                pending = []

                def push_slot(s):
                    tag = f"s{s}"

                    def u_prep():
                        st = {}
                        st["orows"], st["oT"] = sm_prep(
                            s, tok_i[:, s:s + 1], tag)
                        slot_state[tag] = st

                    def u_head():
                        st = slot_state[tag]
                        sm_head(st["oT"], hsums[:, s, :], hsr[:, s:s + 1], tag)
                        sm_xhd(st["orows"], hd_i[:, s:s + 1], xhd[:, s:s + 1],
                               tag)

                    def u_tailprep():
                        st = slot_state[tag]
                        st["prow"], st["pT"] = sm_tailprep(st["oT"], tag)

                    def u_chunk(g):
                        st = slot_state[tag]
                        sm_chunk(st["pT"], wtail[:, g * 512:(g + 1) * 512],
                                 tsums[:, s, g:g + 1], f"{tag}_{g}")

                    def u_post():
                        st = slot_state[tag]
                        sm_tailpost(tsums[:, s, :], tsr[:, s:s + 1],
                                    st["prow"], tl_i[:, s:s + 1],
                                    xtl[:, s:s + 1], tag)

                    pending.append((1, u_prep))
                    pending.append((3, u_head))
                    pending.append((1, u_tailprep))
                    for g in range(TPAD // 512):
                        pending.append((1, lambda g=g: u_chunk(g)))
                    pending.append((1, u_post))

                def push_ep(e):
                    tag = f"e{e}"

                    def u_prep():
                        st = {}
                        st["orows"], st["oT"] = sm_prep(
                            None, ep_tok[:, e:e + 1], tag)
                        slot_state[tag] = st

                    def u_head():
                        st = slot_state[tag]
                        sm_head(st["oT"], ep_hsums[:, e, :],
                                ep_hsr[:, e:e + 1], tag)
                        sm_xhd(st["orows"], ep_hd[:, e:e + 1],
                               ep_hx[:, NEP + e:NEP + e + 1], tag)

                    def u_tailprep():
                        st = slot_state[tag]
                        st["prow"], st["pT"] = sm_tailprep(st["oT"], tag)

                    def u_chunk(j):
                        st = slot_state[tag]
                        sm_chunk(st["pT"], wtail_ep[:, j * 512:(j + 1) * 512],
                                 ep_tsums[:, e, j:j + 1], f"{tag}_{j}")

                    def u_post():
                        st = slot_state[tag]
                        sm_tailpost(ep_tsums[:, e, :], ep_ts[:, e:e + 1],
                                    st["prow"], ep_tl[:, e:e + 1],
                                    ep_hx[:, 2 * NEP + e:2 * NEP + e + 1],
                                    tag)

                    pending.append((1, u_prep))
                    pending.append((3, u_head))
                    pending.append((1, u_tailprep))
                    for j in range(EPW // 512):
                        pending.append((1, lambda j=j: u_chunk(j)))
                    pending.append((1, u_post))

                def drain(budget):
                    spent = 0
                    while pending and spent < budget:
                        cost, fn = pending.pop(0)
                        fn()
                        spent += cost

                # slot s firable after iteration 2*max_tile(s)+3
                slot_fire = {0: 17, 1: 33, 2: 43}
                ep_fire = {0: 45, 1: 47, 2: 49, 3: 10 ** 9}

                # ---------------- main pipelined loop -----------------------
                emit_emb_tile(0)
                emit_emb_tile(1)

                ot_pend = {}
                h1hist = {-1: h1}
                h2hist = {-2: h2, -1: h2}

                def rhs_l1g(t):
                    def f(k):
                        if k < 2:
                            return embT[:, k, ts(t, 64)]
                        return h1hist[t - 1][:, k - 2, :]
                    return f

                def rhs_l1c(t, rh):
                    def f(k):
                        if k < 2:
                            return embT[:, k, ts(t, 64)]
                        return rh[:, k - 2, :]
                    return f

                def rhs_l2g(t):
                    def f(k):
                        if k < 8:
                            return h1hist[t][:, k, :]
                        return h2hist[t - 1][:, k - 8, :]
                    return f

                def rhs_l2c(t, rh):
                    def f(k):
                        if k < 8:
                            return h1hist[t][:, k, :]
                        return rh[:, k - 8, :]
                    return f

                for t in range(T):
                    if t % 2 == 0 and (t // 2 + 2) < NTILE:
                        emit_emb_tile(t // 2 + 2)
                    if t == 2:
                        load_sm_weights()

                    # A1: L1 gates matmul for step t
                    pg1 = gates_mm(wg1, KG1, rhs_l1g(t), f"l1_{t}")
                    # A2: L2 gates matmul for step t-1
                    pg2 = None
                    if t >= 1:
                        pg2 = gates_mm(wg2, KG2, rhs_l2g(t - 1), f"l2_{t - 1}")
                    # B-r: r-half evac + sigmoid + r*h (feeds candidates)
                    rh1 = gates_post_r(pg1, bg1, h1hist[t - 1], f"l1_{t}")
                    rh2 = None
                    if t >= 1:
                        rh2 = gates_post_r(pg2, bg2, h2hist[t - 2],
                                           f"l2_{t - 1}")
                    # B-u: u-half evac + sigmoid (feeds h updates)
                    gu1 = gates_post_u(pg1, bg1, f"l1_{t}")
                    gu2 = None
                    if t >= 1:
                        gu2 = gates_post_u(pg2, bg2, f"l2_{t - 1}")
                    # E1: projection matmul + evac for step t-2 (vector work
                    # lands early so the later transposes never block the PE)
                    if t >= 2:
                        ot_pend[t - 2] = emit_proj_mm(t - 2, h2hist[t - 2])
                    # C1 + D1: L1 candidate, h1(t)
                    pc1 = cand_mm(wc1, KG1, rhs_l1c(t, rh1), f"l1_{t}")
                    h1hist[t] = cand_post(pc1, bc1, gu1, h1hist[t - 1], "h1")
                    # C2 + D2: L2 candidate, h2(t-1)
                    if t >= 1:
                        pc2 = cand_mm(wc2, KG2, rhs_l2c(t - 1, rh2),
                                      f"l2_{t - 1}")
                        h2hist[t - 1] = cand_post(pc2, bc2, gu2,
                                                  h2hist[t - 2], "h2")
                    # E2: transposes + orow DMA for step t-2
                    if t >= 2:
                        emit_proj_tr(t - 2, ot_pend.pop(t - 2))
                        del h2hist[t - 2]
                    del h1hist[t - 1]

                    # F: paced softmax work
                    for s in range(NSLOT):
                        if slot_fire.get(s) == t:
                            push_slot(s)
                    for e in range(NEP):
                        if ep_fire.get(e) == t:
                            push_ep(e)
                    drain(CHUNK_BUDGET if t < 40 else 16)

                # ---- loop tail: L2(T-1), proj(T-2), proj(T-1) --------------
                pg2 = gates_mm(wg2, KG2, rhs_l2g(T - 1), f"l2_{T - 1}")
                rh2 = gates_post_r(pg2, bg2, h2hist[T - 2], f"l2_{T - 1}")
                gu2 = gates_post_u(pg2, bg2, f"l2_{T - 1}")
                pc2 = cand_mm(wc2, KG2, rhs_l2c(T - 1, rh2), f"l2_{T - 1}")
                h2hist[T - 1] = cand_post(pc2, bc2, gu2, h2hist[T - 2], "h2")
                ot_a = emit_proj_mm(T - 2, h2hist[T - 2])
                emit_proj_tr(T - 2, ot_a)
                ot_b = emit_proj_mm(T - 1, h2hist[T - 1])
                emit_proj_tr(T - 1, ot_b)
                push_ep(3)
                drain(10 ** 9)

                # ---- batched Ln over all slot/epilogue sums ----------------
                nc.scalar.activation(out=lzh[:], in_=hsr[:], func=AF.Ln,
                                     bias=hpadc[:, 0:1])
                nc.scalar.activation(out=lzt[:], in_=tsr[:], func=AF.Ln,
                                     bias=tpadc[:, 0:1])
                nc.scalar.activation(out=ep_hx[:, 0:NEP], in_=ep_hsr[:],
                                     func=AF.Ln, bias=hpadc[:, 0:1])

                # ---- loss assembly ----------------------------------------
                d3 = SW.tile([128, NSLOT], F32, tag="d3")
                nc.vector.tensor_sub(out=d3[:], in0=lzh[:], in1=xhd[:])
                d1 = SW.tile([128, NSLOT], F32, tag="d1")
                nc.vector.tensor_sub(out=d1[:], in0=lzt[:], in1=xtl[:])
                d2 = SW.tile([128, NSLOT], F32, tag="d2")
                nc.vector.tensor_mul(out=d2[:], in0=d1[:], in1=mt_m[:])
                d4 = SW.tile([128, NSLOT], F32, tag="d4")
                nc.vector.tensor_add(out=d4[:], in0=d3[:], in1=d2[:])
                nc.vector.tensor_mul(out=loss_t[:], in0=d4[:], in1=vl_m[:])
                lv = SW.tile([128, 1], F32, tag="lv")
                nc.vector.tensor_reduce(
                    out=lv[:], in_=loss_t[:], op=AL.add,
                    axis=mybir.AxisListType.X)
                pl = PP.tile([128, 512], F32, tag="smb", space="PSUM")
                nc.tensor.matmul(out=pl[0:1, 0:1], lhsT=lv[:], rhs=ones[:],
                                 start=True, stop=True)
                lsb = SW.tile([1, 1], F32, tag="lsb")
                nc.vector.tensor_copy(out=lsb[:], in_=pl[0:1, 0:1])
                nc.sync.dma_start(out=loss_e[:], in_=lsb[:])
                nc.sync.dma_start(out=eptsum_e[:], in_=ep_ts[:])
                nc.sync.dma_start(out=ephx_e[:], in_=ep_hx[:])

    nc.compile()
    return nc


def prep_inputs(input_data, targets, embedding, Wg1, bg1, Wc1, bc1, Wg2, bg2,
                Wc2, bc2, Wp, bp, W_head, W_tp, W_tail):
    bf = ml_dtypes.bfloat16
    f8 = ml_dtypes.float8_e4m3fn

    def ktile(w, kt, n, dt=bf, scale=1.0):
        return np.ascontiguousarray(
            (w * scale).reshape(kt, 128, n).transpose(1, 0, 2)).astype(dt)

    def biasT(b, m):
        return np.ascontiguousarray(b.reshape(m, 128).T).astype(np.float32)

    ids_t = np.ascontiguousarray(input_data.T).reshape(-1).astype(np.int32)
    shared = {
        "ids_sb": np.ascontiguousarray(ids_t.reshape(NTILE, 128).T),
        "emb": embedding.astype(bf),
        "wg1": ktile(Wg1, KG1, 2 * R, f8, WSCALE),
        "wc1": ktile(Wc1, KG1, R, f8, WSCALE),
        "wg2": ktile(Wg2, KG2, 2 * R, f8, WSCALE),
        "wc2": ktile(Wc2, KG2, R, f8, WSCALE),
        "wp": ktile(Wp, R // 128, U),
        "bg1": biasT(bg1, 16), "bc1": biasT(bc1, 8),
        "bg2": biasT(bg2, 16), "bc2": biasT(bc2, 8), "bp": biasT(bp, 2),
    }
    whead_p = np.zeros((U, HPAD), np.float32)
    whead_p[:, :CUT + 1] = W_head
    shared["whead"] = ktile(whead_p, 2, HPAD)
    wtail_p = np.zeros((TAILP, TPAD), np.float32)
    wtail_p[:, :V - CUT] = W_tail
    shared["wtail"] = wtail_p.astype(f8)
    wtail_pep = np.zeros((TAILP, TPAD_EP), np.float32)
    wtail_pep[:, :V - CUT] = W_tail
    wtail_pep = wtail_pep.astype(f8)
    shared["wtp"] = ktile(W_tp.astype(np.float32), 2, TAILP)
    shared["wheadT"] = np.ascontiguousarray(W_head.T).astype(np.float32)
    shared["wtailT"] = np.ascontiguousarray(W_tail.T).astype(np.float32)

    tgt_t = np.ascontiguousarray(targets.T).reshape(-1).astype(np.int64)

    # shared epilogue arrays (tiles 21..24)
    ep_tok = np.zeros((128, NEP), np.int32)
    ep_hd = np.zeros((128, NEP), np.int32)
    ep_tl = np.zeros((128, NEP), np.int32)
    for e in range(NEP):
        tile_idx = EP_T0 + e
        toks = np.arange(tile_idx * 128, (tile_idx + 1) * 128)
        tg = tgt_t[toks]
        ep_tok[:, e] = toks
        ep_hd[:, e] = np.minimum(tg, CUT)
        ep_tl[:, e] = np.clip(tg - CUT, 0, V - CUT - 1)
    shared["ep_tok"] = ep_tok
    shared["ep_hd"] = ep_hd
    shared["ep_tl"] = ep_tl

    per_core = []
    for c in range(NCORES):
        tok = np.zeros((128, NSLOT), np.int32)
        hdi = np.zeros((128, NSLOT), np.int32)
        tli = np.zeros((128, NSLOT), np.int32)
        mtl = np.zeros((128, NSLOT), np.float32)
        vld = np.zeros((128, NSLOT), np.float32)
        for s in range(NSLOT):
            tile_idx = 8 * s + c
            if tile_idx >= EP_T0:
                continue
            toks = np.arange(tile_idx * 128, (tile_idx + 1) * 128)
            tg = tgt_t[toks]
            tok[:, s] = toks
            hdi[:, s] = np.minimum(tg, CUT)
            tli[:, s] = np.clip(tg - CUT, 0, V - CUT - 1)
            mtl[:, s] = (tg >= CUT)
            vld[:, s] = 1.0
        per_core.append({
            "tok_idx": tok, "hd_idx": hdi, "tl_idx": tli,
            "mtail": mtl, "valid": vld,
            "wtail_ep": np.ascontiguousarray(
                wtail_pep[:, c * EPW:(c + 1) * EPW]),
        })
    return shared, per_core, tgt_t


_CACHE = {}


def kernel(**inputs):
    import os
    if "prog" not in _CACHE:
        _CACHE["prog"] = build_program()
    nc = _CACHE["prog"]
    shared, per_core, tgt_t = prep_inputs(**{
        k: np.asarray(inputs[k]) for k in (
            "input_data", "targets", "embedding", "Wg1", "bg1", "Wc1", "bc1",
            "Wg2", "bg2", "Wc2", "bc2", "Wp", "bp", "W_head", "W_tp", "W_tail")})
    in_maps = [dict(shared, **pc) for pc in per_core]
    trace = bool(int(os.environ.get("KERNEL_TRACE", "0")))
    res = run_bass_kernel_spmd(nc, in_maps, core_ids=list(range(NCORES)),
                               trace=trace)
    if trace:
        kernel.last_exec_time_ns = res.exec_time_ns

    total = sum(float(res.results[c]["loss_sum"][0, 0]) for c in range(NCORES))
    # epilogue tiles 21..24: combine per-core partial tail sums on the host
    ep_hx = res.results[0]["ep_hx"].astype(np.float64)
    tsum = np.zeros((128, NEP), np.float64)
    for c in range(NCORES):
        tsum += res.results[c]["ep_tsum"].astype(np.float64)
    tsum -= float(TPAD_EP - (V - CUT))  # padded columns contribute exp(0)=1
    lzt_ep = np.log(tsum)
    for e in range(NEP):
        toks = np.arange((EP_T0 + e) * 128, (EP_T0 + e + 1) * 128)
        tg = tgt_t[toks]
        mask = (tg >= CUT).astype(np.float64)
        lzh_e = ep_hx[:, e]
        xhd_e = ep_hx[:, NEP + e]
        xtl_e = ep_hx[:, 2 * NEP + e]
        total += float(np.sum((lzh_e - xhd_e) + mask * (lzt_ep[:, e] - xtl_e)))
    return np.float32(total / NT)
